# revision 10
# baseline (speedup 1.0000x reference)
"""Trainium2 kernel for nn_CrossAttMultiplexer.

Reference math:
    q = x_r @ WQ ; k = s_r @ WK ; v = s_r @ WV      (per-pixel, c=96 "tokens", feat dim 1)
    scores[n,i,j] = (q.k)/sqrt(d) = g * x[n,i] * s[n,j]   with g = (WQ.WK)/sqrt(d)
    alpha = softmax_j(scores)
    out[n,i] = v[n,i] * sum_j alpha[n,i,j] = v[n,i] * 1 = s[n,i] * WV[0,0]

The softmax rows sum to exactly 1 and v broadcasts over the summed axis, so the
whole module collapses to a single scalar multiply: out = s * WV[0,0].
(Verified vs the fp32 jax reference: max abs err ~1.5e-8.)

Sharding: pure data parallel. The pseudo-batch N = 4*64*64 = 16384 rows of 96
floats is split into 8 contiguous shards; each core views its 768KB shard as
one [128, 1536] f32 tile (a pure reinterpretation of contiguous memory) and
moves column ranges of it. Weights fold into an immediate scalar baked into
the DVE instruction.

Pipeline (raw Bass; TileContext's kernel-tail Drain exceeds the walrus
sync-wait limit on this compile path). The tile is split into 4 column chunks
of 384 (1536B descriptors):
  sync ring   : in0, in2 then out0, out2     (ring qSyncDynamicHW)
  scalar ring : in1, in3 then out1, out3     (ring qScalarDynamicHW)
  vector (DVE): per-chunk tensor_scalar_mul, in-DMA wait fused into the op
  out-DMAs    : scale-gate (v_sem) fused into the DMA instruction itself;
                per-ring FIFO queues their descriptors behind remaining ins
  both engines: final wait on the shared out-completion semaphore

Design space explored on HW (NTFF traces; see session notes):
  * Measured exec window = [first framework MEMSET (~6.4us, unavoidable
    const-init preamble), last instruction end]. The postamble (two engine
    barriers + trace flush, ~7.5us) is FIXED regardless of kernel content
    (verified: near-identical stagger across totally different kernels).
  * Reads are latency-bound: 6144B descriptors -> 139 GB/s, 1536B on one
    queue -> 232 GB/s, 1536B split across BOTH HWDGE rings -> ~330-390 GB/s.
    Writes: 300-380 GB/s. Mixed read/write phases degrade to ~250 GB/s.
  * The in->out chain (completion-sem receipt ~450ns + scale + dispatch
    ~650ns + doorbell ~700ns = ~2.15us) conserves a ~1us handover bubble:
    out descriptors for data landed at t cannot reach the rings before
    t+2.15us. Finer chunking to smooth the lumps LOSES on HW (more
    dispatches/sems: 5-8 chunk variants measured 0.3-3us slower); 4 chunks
    is the sweet spot. 2/1-chunk variants serialize fully (also slower).
  * Also rejected on measurement: gpsimd tensor_scalar for parallel scales
    (~9us slower), gpsimd SWDGE out-dispatch (~0.6us slower), SWDGE
    accum_op=mult inline scaling (backend rejects "mult with Copy mode"),
    max_dma_last_dim splits to <=512B descriptors (RMW penalty).

Correctness notes learned the hard way on HW:
  * then_inc(sem, 16) on a DMA arrives as 16 independent +1s (one per SDMA
    engine), so each in-DMA gets a DEDICATED semaphore.
  * The final wait_ge on the out-DMA completion sem is REQUIRED; relying on
    the block-exit drain intermittently returns stale output.
"""

from contextlib import ExitStack

import numpy as np

# Full-problem constants (hardcoded per harness contract).
B, H, W, C = 4, 64, 64, 96
N_CORES = 8
P = 128                                # SBUF partitions
F = (B * H * W * C) // (N_CORES * P)   # 1536 floats per partition per core
WIDTHS = (384, 384, 384, 384)          # column chunks of the [128, 1536] tile

_PROG_CACHE: dict = {}


def _build_program(wv: float):
    import concourse.bass as bass
    from concourse import mybir

    f32 = mybir.dt.float32
    nt = len(WIDTHS)
    offs = np.cumsum([0] + list(WIDTHS))

    nc = bass.Bass()
    s_in = nc.declare_dram_parameter("s_shard", [P, F], f32, isOutput=False)
    out_ext = nc.declare_dram_parameter("out", [P, F], f32, isOutput=True)

    with ExitStack() as ctx:
        block = ctx.enter_context(nc.Block())
        isems = [ctx.enter_context(nc.semaphore(f"in{i}")) for i in range(nt)]
        v_sem = ctx.enter_context(nc.semaphore("v_sem"))
        o_sem = ctx.enter_context(nc.semaphore("o_sem"))
        in_buf = ctx.enter_context(nc.sbuf_tensor("in_buf", [P, F], f32))
        out_buf = ctx.enter_context(nc.sbuf_tensor("out_buf", [P, F], f32))

        def cols(t, i):
            return t[:, int(offs[i]):int(offs[i + 1])]

        def engine_prog(eng, chunks):
            for i in chunks:
                eng.dma_start(out=cols(in_buf, i), in_=cols(s_in, i)).then_inc(
                    isems[i], 16
                )
            for i in chunks:
                eng.dma_start(out=cols(out_ext, i), in_=cols(out_buf, i))._wait_ge(
                    v_sem, i + 1
                ).then_inc(o_sem, 16)
            eng.wait_ge(o_sem, 16 * nt)

        @block.sync
        def _(sync):
            engine_prog(sync, [0, 2])

        @block.scalar
        def _(scalar):
            engine_prog(scalar, [1, 3])

        @block.vector
        def _(vector):
            for i in range(nt):
                vector.tensor_scalar_mul(
                    cols(out_buf, i), cols(in_buf, i), wv
                )._wait_ge(isems[i], 16).then_inc(v_sem, 1)

    return nc


def _get_program(wv: float):
    key = np.float32(wv).tobytes()
    if key not in _PROG_CACHE:
        _PROG_CACHE[key] = _build_program(wv)
    return _PROG_CACHE[key]


def _run(x, s, WQ, WK, WV, trace: bool = False):
    from concourse.bass_utils import run_bass_kernel_spmd

    s = np.ascontiguousarray(np.asarray(s, dtype=np.float32))
    wv = float(np.asarray(WV, dtype=np.float32).reshape(-1)[0])

    shards = s.reshape(N_CORES, P, F)
    in_maps = [{"s_shard": shards[i]} for i in range(N_CORES)]

    nc = _get_program(wv)
    res = run_bass_kernel_spmd(nc, in_maps, list(range(N_CORES)), trace=trace)
    out = np.stack([np.asarray(res.results[i]["out"]) for i in range(N_CORES)])
    return out.reshape(B, H, W, C).astype(np.float32, copy=False), res


def kernel(x, s, WQ, WK, WV):
    out, _ = _run(x, s, WQ, WK, WV)
    return out
